# revision 1
# baseline (speedup 1.0000x reference)
"""DeepRNN (3-layer, relu+tanh+tanh) Trainium2 kernel.

Strategy: data-parallel over batch (64 -> 8 cores x 8). Each core runs the
full 3-layer network on its batch slice.

Per layer:
  - Input projection xp = prev @ WihT + (bih+bhh) is computed in 16-step row
    chunks (M=128 rows = 16 steps x 8 batch) on the PE, software-pipelined
    one chunk ahead of the scan.
  - The scan step computes psum = xp_t (injected via identity matmul) +
    h_{t-1} @ WhhT (stationary = h^T columns, moving = WhhT rows, fp32r at
    N=512 -> 1 cycle/row), then act() on ScalarE, then rebuilds the h^T
    stationary via 8 PE transposes + one DVE evacuation.

Matmuls run as float32r (TF32-like); xp chunk buffers are bf16 (SBUF fit).
"""

import numpy as np

SEQ = 512
BATCH = 64
D = 1024
NCORES = 8
B = BATCH // NCORES  # 8 rows of batch per core
NK = D // 128  # 8 contraction chunks
CH = 16  # timesteps per chunk
NCH = SEQ // CH  # 32 chunks
ROWS = SEQ * B  # 4096

_BUILD_CACHE = {}


def _make_patched_tc():
    import concourse.tile as tile
    import concourse.mybir as mybir
    from concourse.vector_clock import ScopedClock

    class PatchedTC(tile.TileContext):
        """This walrus build accepts very few sync-wait commands per
        instruction (1 for most structs). Hoist extra waits onto injected
        same-engine nops placed immediately before the offending
        instruction, and split the kernel-tail drain the same way."""

        _WAIT_LIMITS = {}
        _WAIT_DEFAULT = 1

        def _split_waits(self, insts):
            out = []
            for inst in insts:
                si = inst.sync_info
                waits = list(si.on_wait) if si and si.on_wait else []
                limit = self._WAIT_LIMITS.get(type(inst).__name__, self._WAIT_DEFAULT)
                if len(waits) > limit:
                    import concourse.mybir as mybir_

                    keep = waits[:limit]
                    extra = waits[limit:]
                    for w in extra:
                        nop = mybir_.InstNoOp(
                            name=self.nc.get_next_instruction_name(),
                            engine=inst.engine,
                            ins=[],
                            outs=[],
                            sync_info=mybir_.SyncInfo(on_wait=[w], on_update=[]),
                            bass_nofuse=True,
                        )
                        out.append(nop)
                    inst.sync_info = mybir_.SyncInfo(
                        on_wait=keep,
                        on_update=list(si.on_update) if si.on_update else [],
                    )
                out.append(inst)
            return out

        def _lower_ordered_insts(self, postordered_blocks):
            for bb_name in list(postordered_blocks.keys()):
                postordered_blocks[bb_name] = self._split_waits(
                    postordered_blocks[bb_name]
                )
            return super()._lower_ordered_insts(postordered_blocks)

        def _drain_and_barrier(self, tick_clock, wait_clock):
            nc = self.nc
            collector = nc.sync.nop(hint="wait_collector", nofuse=True)
            wait_clock.add_sem_waits(
                collector.ins, ScopedClock({None: tick_clock.global_clock})
            )
            si = collector.ins.sync_info
            waits = list(si.on_wait) if si and si.on_wait else []
            if len(waits) > 1:
                collector.ins.sync_info = mybir.SyncInfo(
                    on_wait=[waits[0]], on_update=[]
                )
                for w in waits[1:]:
                    extra = nc.sync.nop(hint="wait_split", nofuse=True)
                    extra.ins.sync_info = mybir.SyncInfo(on_wait=[w], on_update=[])
            nc.sync.drain()
            nc.all_engine_barrier()
            assert self.sems is not None
            popped = nc._tile_sem_poison_stack.pop()
            assert popped is self._sem_poison
            nc.clear_and_free_semaphores(list(self.sems.allocated().values()))
            nc.all_engine_barrier()

    return PatchedTC


def build(repeat=1):
    if repeat in _BUILD_CACHE:
        return _BUILD_CACHE[repeat]

    import contextlib
    import concourse.bass as bass
    import concourse.tile as tile
    import concourse.mybir as mybir
    from concourse.masks import make_identity

    f32 = mybir.dt.float32
    f32r = mybir.dt.float32r
    bf16 = mybir.dt.bfloat16
    PatchedTC = _make_patched_tc()

    nc = bass.Bass()
    # xT[ch, p, k, r] = x_core[ch*128 + r, k*128 + p]  (transposed input rows)
    xT = nc.dram_tensor("xT", [NCH + 1, 128, NK, 128], bf16, kind="ExternalInput")
    wih = nc.dram_tensor("wihT", [3, D, D], bf16, kind="ExternalInput")
    whh = nc.dram_tensor("whhT", [3, D, D], bf16, kind="ExternalInput")
    bias = nc.dram_tensor("bias", [3, 1, D], bf16, kind="ExternalInput")
    y = nc.dram_tensor("y", [NCH, 128, D], f32, kind="ExternalOutput")
    hdr = nc.dram_tensor("hdr", [2, NCH + 1, 128, D], f32, kind="Internal")

    with PatchedTC(nc) as tc:
        ctx = contextlib.ExitStack()
        with ctx:
            const = ctx.enter_context(tc.tile_pool(name="const", bufs=1))
            wpool = ctx.enter_context(tc.tile_pool(name="wpool", bufs=1))
            xpp = ctx.enter_context(tc.tile_pool(name="xpp", bufs=1))
            work = ctx.enter_context(tc.tile_pool(name="work", bufs=1))
            stg = ctx.enter_context(tc.tile_pool(name="stg", bufs=1))
            pscan = ctx.enter_context(tc.tile_pool(name="pscan", bufs=2, space="PSUM"))
            pst = ctx.enter_context(tc.tile_pool(name="pst", bufs=2, space="PSUM"))
            pproj = ctx.enter_context(tc.tile_pool(name="pproj", bufs=1, space="PSUM"))

            # constants
            I8 = const.tile([8, 8], f32, tag="i8")
            make_identity(nc, I8)
            I8b = const.tile([8, 8], bf16, tag="i8b")
            make_identity(nc, I8b)
            I128 = const.tile([128, 128], f32, tag="i128")
            make_identity(nc, I128)
            ones_f = const.tile([1, 128], f32, tag="ones_f")
            nc.vector.memset(ones_f, 1.0)
            ones = const.tile([1, 128], bf16, tag="ones")
            nc.vector.tensor_copy(out=ones[:, :], in_=ones_f[:, :])
            zt = const.tile([128, D], f32, tag="zt")
            nc.vector.memset(zt, 0.0)
            # zero the pad chunk of both intermediate layer outputs
            nc.sync.dma_start(out=hdr[0, NCH, :, :], in_=zt[:, :])
            nc.sync.dma_start(out=hdr[1, NCH, :, :], in_=zt[:, :])

            def emit_phase(l, act_func):
                wih_sb = wpool.tile([128, NK * D], bf16, tag="wih")
                whh_sb = wpool.tile([128, NK * D], bf16, tag="whh")
                bias_sb = wpool.tile([1, D], bf16, tag="bias")
                for k in range(NK):
                    nc.sync.dma_start(
                        out=wih_sb[:, k * D : (k + 1) * D],
                        in_=wih[l, k * 128 : (k + 1) * 128, :],
                    )
                    nc.sync.dma_start(
                        out=whh_sb[:, k * D : (k + 1) * D],
                        in_=whh[l, k * 128 : (k + 1) * 128, :],
                    )
                nc.sync.dma_start(out=bias_sb[:, :], in_=bias[l, :, :])

                hT = wpool.tile([128, B * NK], bf16, tag="hT")
                nc.vector.tensor_copy(out=hT[:, :], in_=zt[:, 0 : B * NK])
                xpA = xpp.tile([B, 8 * D], bf16, tag="xpA")
                xpB = xpp.tile([B, 8 * D], bf16, tag="xpB")

                def emit_proj(ch):
                    """Emit input-projection MMs for row chunk `ch` (python
                    int or ScalarValue). Returns the proj psum tile."""
                    oT = stg.tile([128, NK, 128], bf16, tag="oT")
                    chs = ch if isinstance(ch, int) else bass.ds(ch, 1)
                    if l == 0:
                        nc.sync.dma_start(out=oT[:, :, :], in_=xT[chs, :, :, :])
                    else:
                        hrows = stg.tile([128, D], f32, tag="hrows")
                        nc.sync.dma_start(
                            out=hrows[:, :], in_=hdr[l - 1, chs, :, :]
                        )
                        for k in range(NK):
                            ptr = pst.tile([128, 128], f32, tag="pst")
                            nc.tensor.transpose(
                                out=ptr[:, :],
                                in_=hrows[:, k * 128 : (k + 1) * 128],
                                identity=I128[:, :],
                            )
                            nc.vector.tensor_copy(out=oT[:, k, :], in_=ptr[:, :])
                    pp = pproj.tile([128, D], f32, tag="pp")
                    for h in range(2):
                        sl = slice(h * 512, (h + 1) * 512)
                        nc.tensor.matmul(
                            pp[:, sl],
                            lhsT=ones[:, :],
                            rhs=bias_sb[:, sl],
                            start=True,
                            stop=False,
                        )
                        for k in range(NK):
                            nc.tensor.matmul(
                                pp[:, sl],
                                lhsT=oT[:, k, :],
                                rhs=wih_sb[:, k * D + h * 512 : k * D + h * 512 + 512],
                                start=False,
                                stop=(k == NK - 1),
                            )
                    return pp

                def evac_proj(pp):
                    st = stg.tile([128, D], bf16, tag="pstage")
                    nc.vector.tensor_copy(out=st[:, :], in_=pp[:, :])
                    return st

                def remap(st, half, xp):
                    # staging rows half*64+s*8 .. +8 -> xp[:, s*D:(s+1)*D]
                    for s in range(8):
                        r0 = half * 64 + s * 8
                        nc.sync.dma_start(
                            out=xp[:, s * D : (s + 1) * D],
                            in_=st[r0 : r0 + 8, :],
                        )

                def scan_step(iv, tl, xp, h_acc):
                    """One timestep; xp holds this step's projection at slot
                    tl%8; activations land in h_acc[:, tl, :]."""
                    s = tl % 8
                    ps0 = pscan.tile([B, 512], f32, tag="ps0")
                    ps1 = pscan.tile([B, 512], f32, tag="ps1")
                    for h, ps in ((0, ps0), (1, ps1)):
                        nc.tensor.matmul(
                            ps[:, :],
                            lhsT=I8b[:, :],
                            rhs=xp[:, s * D + h * 512 : s * D + h * 512 + 512],
                            start=True,
                            stop=False,
                        )
                        for k in range(NK):
                            nc.tensor.matmul(
                                ps[:, :],
                                lhsT=hT[:, k * B : (k + 1) * B],
                                rhs=whh_sb[:, k * D + h * 512 : k * D + h * 512 + 512],
                                start=False,
                                stop=(k == NK - 1),
                            )
                    nc.scalar.activation(h_acc[:, tl, 0:512], ps0[:, :], act_func)
                    nc.scalar.activation(h_acc[:, tl, 512:1024], ps1[:, :], act_func)
                    # rebuild transposed state
                    pT = pst.tile([128, B * NK], f32, tag="pst")
                    for k in range(NK):
                        nc.tensor.transpose(
                            out=pT[:, k * B : (k + 1) * B],
                            in_=h_acc[:, tl, k * 128 : (k + 1) * 128],
                            identity=I8[:, :],
                        )
                    nc.vector.tensor_copy(out=hT[:, :], in_=pT[:, :])

                def store_chunk(iv, h_acc):
                    # h_acc [B, CH, D] -> dest chunk [(t b), d] reordered
                    if l == 2:
                        dst = y[bass.ds(iv, 1), :, :]
                    else:
                        dst = hdr[l, bass.ds(iv, 1), :, :]
                    dst = dst.rearrange("a (t b) d -> a b t d", b=B)
                    nc.sync.dma_start(out=dst, in_=h_acc[:, :, :])

                # prefill chunk 0
                pp = emit_proj(0)
                st = evac_proj(pp)
                remap(st, 0, xpA)
                remap(st, 1, xpB)

                with tc.For_i(
                    0, NCH, 1, hint_engines=(mybir.EngineType.PE,)
                ) as iv:
                    h_acc = work.tile([B, CH, D], f32, tag="hacc")
                    pp = emit_proj(iv + 1)
                    for tl in range(8):
                        scan_step(iv, tl, xpA, h_acc)
                    st = evac_proj(pp)
                    remap(st, 0, xpA)
                    for tl in range(8, 16):
                        scan_step(iv, tl, xpB, h_acc)
                    remap(st, 1, xpB)
                    store_chunk(iv, h_acc)

            if repeat == 1:
                for l in range(3):
                    emit_phase(l, _act_for_layer(l))
                    tc.strict_bb_all_engine_barrier()
            else:
                # runtime repetition of the whole net, for timing via
                # wall-clock difference against repeat=1
                with tc.For_i(0, repeat, 1) as _rep:
                    for l in range(3):
                        emit_phase(l, _act_for_layer(l))
                        tc.strict_bb_all_engine_barrier()

    _BUILD_CACHE[repeat] = nc
    return nc


def _act_for_layer(l):
    import concourse.mybir as mybir

    return (
        mybir.ActivationFunctionType.Relu
        if l == 0
        else mybir.ActivationFunctionType.Tanh
    )


def _prep_inputs(x, wihT, whhT, bias2):
    """Build per-core in_maps (bf16 device inputs)."""
    import ml_dtypes

    bf = ml_dtypes.bfloat16
    wihT = wihT.astype(bf)
    whhT = whhT.astype(bf)
    bias2 = bias2.astype(bf)
    in_maps = []
    for c in range(NCORES):
        xc = x[:, c * B : (c + 1) * B, :].reshape(ROWS, D)  # [rows, din]
        # xT[ch, p, k, r] = xc[ch*128 + r, k*128 + p]
        xTc = np.zeros((NCH + 1, 128, NK, 128), dtype=bf)
        xTc[:NCH] = np.ascontiguousarray(
            xc.reshape(NCH, 128, NK, 128).transpose(0, 3, 2, 1)
        ).astype(bf)
        in_maps.append({"xT": xTc, "wihT": wihT, "whhT": whhT, "bias": bias2})
    return in_maps


def kernel(
    x,
    Wih0,
    Whh0,
    bih0,
    bhh0,
    Wih1,
    Whh1,
    bih1,
    bhh1,
    Wih2,
    Whh2,
    bih2,
    bhh2,
):
    from concourse import bass_utils

    x = np.asarray(x, dtype=np.float32)
    wihT = np.ascontiguousarray(
        np.stack([np.asarray(w, np.float32).T for w in (Wih0, Wih1, Wih2)])
    )
    whhT = np.ascontiguousarray(
        np.stack([np.asarray(w, np.float32).T for w in (Whh0, Whh1, Whh2)])
    )
    bias2 = np.ascontiguousarray(
        np.stack(
            [
                (np.asarray(a, np.float32) + np.asarray(b, np.float32))[None, :]
                for a, b in ((bih0, bhh0), (bih1, bhh1), (bih2, bhh2))
            ]
        )
    )
    nc = build(repeat=1)
    in_maps = _prep_inputs(x, wihT, whhT, bias2)
    res = bass_utils.run_bass_kernel_spmd(
        nc, in_maps, core_ids=list(range(NCORES)), trace=False
    )
    out = np.empty((SEQ, BATCH, D), dtype=np.float32)
    for c in range(NCORES):
        out[:, c * B : (c + 1) * B, :] = res.results[c]["y"].reshape(SEQ, B, D)
    return out



# revision 21
# speedup vs baseline: 6.1921x; 6.1921x over previous
"""DeepRNN (3-layer, relu+tanh+tanh) Trainium2 kernel, v3.

Strategy: data-parallel over batch (64 -> 8 cores x 8 rows). The three
layers' time-scans run INTERLEAVED in one loop, skewed by one 16-step
chunk (at iteration i: layer 0 scans chunk i, layer 1 chunk i-1, layer 2
chunk i-2): while one layer's activation/transpose chain drains, the
other layers' matmuls keep the PE busy.

The input projection leaves xp for all 16 steps of a chunk RESIDENT IN
PSUM (rows = (t, b), has_written set), and each step's recurrent matmuls
accumulate h_{t-1} @ Whh^T directly onto it -- no xp evacuation, no
injection matmuls. Matmul outputs must start at 32-aligned partitions,
so the stationary h^T lives in 32-wide zero-padded slots with the valid
8 columns at offset ((t+1)%4)*8: the matmul writes the aligned 32-row
group containing step t+1's rows, and the zero columns accumulate 0
onto the neighbouring steps' rows. Activations likewise process the
full aligned 32-row group (same per-lane cost); the per-step h^T is
extracted by transpose-mode matmuls whose moving operand is a
shifted-masked selector (valid rows shifted to the next slot offset,
garbage rows dropped). Transposed state in SBUF rings feeds both the
recurrent stationary and, windowed per chunk parity, the next layer's
projection stationary -- no DRAM round-trips inside the loop.
"""

import numpy as np

SEQ = 512
BATCH = 64
D = 1024
NCORES = 8
B = BATCH // NCORES  # 8 batch rows per core
CH = 16  # timesteps per chunk
NCH = SEQ // CH  # 32 chunks

FP8 = False
WSCALE = 256.0

_BUILD_CACHE = {}


def _make_patched_tc():
    import concourse.tile as tile
    import concourse.mybir as mybir
    from concourse.vector_clock import ScopedClock

    class PatchedTC(tile.TileContext):
        """This walrus build accepts very few sync-wait commands per
        instruction (1 for most structs). Hoist extra waits onto injected
        same-engine nops placed immediately before the offending
        instruction, and split the kernel-tail drain the same way."""

        _WAIT_LIMITS = {}
        _WAIT_DEFAULT = 1

        def _split_waits(self, insts):
            out = []
            for inst in insts:
                si = inst.sync_info
                waits = list(si.on_wait) if si and si.on_wait else []
                limit = self._WAIT_LIMITS.get(type(inst).__name__, self._WAIT_DEFAULT)
                if len(waits) > limit:
                    import concourse.mybir as mybir_

                    keep = waits[:limit]
                    extra = waits[limit:]
                    for w in extra:
                        nop = mybir_.InstNoOp(
                            name=self.nc.get_next_instruction_name(),
                            engine=inst.engine,
                            ins=[],
                            outs=[],
                            sync_info=mybir_.SyncInfo(on_wait=[w], on_update=[]),
                            bass_nofuse=True,
                        )
                        out.append(nop)
                    inst.sync_info = mybir_.SyncInfo(
                        on_wait=keep,
                        on_update=list(si.on_update) if si.on_update else [],
                    )
                out.append(inst)
            return out

        def _lower_ordered_insts(self, postordered_blocks):
            for bb_name in list(postordered_blocks.keys()):
                postordered_blocks[bb_name] = self._split_waits(
                    postordered_blocks[bb_name]
                )
            return super()._lower_ordered_insts(postordered_blocks)

        def _drain_and_barrier(self, tick_clock, wait_clock):
            nc = self.nc
            collector = nc.sync.nop(hint="wait_collector", nofuse=True)
            wait_clock.add_sem_waits(
                collector.ins, ScopedClock({None: tick_clock.global_clock})
            )
            si = collector.ins.sync_info
            waits = list(si.on_wait) if si and si.on_wait else []
            if len(waits) > 1:
                collector.ins.sync_info = mybir.SyncInfo(
                    on_wait=[waits[0]], on_update=[]
                )
                for w in waits[1:]:
                    extra = nc.sync.nop(hint="wait_split", nofuse=True)
                    extra.ins.sync_info = mybir.SyncInfo(on_wait=[w], on_update=[])
            nc.sync.drain()
            nc.all_engine_barrier()
            assert self.sems is not None
            popped = nc._tile_sem_poison_stack.pop()
            assert popped is self._sem_poison
            nc.clear_and_free_semaphores(list(self.sems.allocated().values()))
            nc.all_engine_barrier()

    return PatchedTC


def build(repeat=1, fp8=None, unroll=False):
    if fp8 is None:
        fp8 = FP8
    key = (repeat, fp8, unroll)
    if key in _BUILD_CACHE:
        return _BUILD_CACHE[key]

    import contextlib
    import concourse.bass as bass
    import concourse.tile as tile
    import concourse.mybir as mybir
    from concourse.masks import make_identity

    f32 = mybir.dt.float32
    bf16 = mybir.dt.bfloat16
    f8 = mybir.dt.float8e4
    wdt = f8 if fp8 else bf16  # weight / transposed-state dtype
    KC = 4 if fp8 else 8  # contraction chunks per group
    O = 2 if fp8 else 1  # DoubleRow sub-rows per partition
    DR = mybir.MatmulPerfMode.DoubleRow if fp8 else None
    scale = 1.0 / WSCALE if fp8 else 1.0
    PatchedTC = _make_patched_tc()
    ACT = mybir.ActivationFunctionType

    nc = bass.Bass()
    # xT[ch, p, o, kc, t, b] = x[ch*CH+t, b, kc*128*O + o*128 + p]
    xT = nc.dram_tensor("xT", [NCH, 128, O, KC, CH, B], wdt, kind="ExternalInput")
    # w[l, p, kc, o, j] = S * W_l[j, kc*128*O + o*128 + p]
    wih = nc.dram_tensor("wihT", [3, 128, KC, O, D], wdt, kind="ExternalInput")
    whh = nc.dram_tensor("whhT", [3, 128, KC, O, D], wdt, kind="ExternalInput")
    bias = nc.dram_tensor("bias", [3, 1, D], bf16, kind="ExternalInput")
    ishd = nc.dram_tensor("ishift", [32, 4, 32], bf16, kind="ExternalInput")
    y = nc.dram_tensor("y", [NCH, CH, B, D], bf16, kind="ExternalOutput")

    with PatchedTC(nc) as tc:
        ctx = contextlib.ExitStack()
        with ctx:
            const = ctx.enter_context(tc.tile_pool(name="const", bufs=1))
            wpool = ctx.enter_context(tc.tile_pool(name="wpool", bufs=1))
            ring = ctx.enter_context(tc.tile_pool(name="ring", bufs=1))
            hp = ctx.enter_context(tc.tile_pool(name="hp", bufs=2))
            ypool = ctx.enter_context(tc.tile_pool(name="ypool", bufs=1))
            pst = ctx.enter_context(tc.tile_pool(name="pst", bufs=2, space="PSUM"))
            pproj = ctx.enter_context(tc.tile_pool(name="pproj", bufs=1, space="PSUM"))

            ones = const.tile([1, 128], bf16, tag="ones")
            nc.vector.memset(ones, 1.0)
            # ishift[:, r, :]: [32,32] selector: row k=8r+b -> col (k+8)%32,
            # all other rows dropped (zero columns out of the transpose)
            ish = const.tile([32, 4, 32], bf16, tag="ish")
            nc.sync.dma_start(out=ish[:, :, :], in_=ishd[:, :, :])

            wih_sb, whh_sb, bias_sb, scanring, projring = [], [], [], [], []
            for l in range(3):
                wih_sb.append(wpool.tile([128, KC, O, D], wdt, tag=f"wih{l}", name=f"wih{l}"))
                whh_sb.append(wpool.tile([128, KC, O, D], wdt, tag=f"whh{l}", name=f"whh{l}"))
                bias_sb.append(wpool.tile([1, D], bf16, tag=f"bias{l}", name=f"bias{l}"))
                nc.sync.dma_start(out=wih_sb[l][:, :, :, :], in_=wih[l, :, :, :, :])
                nc.sync.dma_start(out=whh_sb[l][:, :, :, :], in_=whh[l, :, :, :, :])
                nc.sync.dma_start(out=bias_sb[l][:, :], in_=bias[l, :, :])
                # scanring[l][p, t, o, kc, b] = h_l[b, kc*128*O+o*128+p] @ step t
                sr = ring.tile([128, CH, O, KC, 32], wdt, tag=f"sring{l}")
                nc.vector.memset(sr, 0.0)
                scanring.append(sr)
                if l < 2:
                    # projring[l][p, o, kc, w, t, b]; window w = chunk % 2
                    projring.append(
                        ring.tile(
                            [128, O, KC, 2, CH, B],
                            wdt,
                            tag=f"pring{l}",
                            name=f"pring{l}",
                        )
                    )
            xTring = ring.tile([128, O, KC, 2, CH, B], wdt, tag="xring")

            def chunk_idx(c):
                """c = int or (iv, add) -> (DRAM index expr, window parity)."""
                if isinstance(c, tuple):
                    iv, add = c
                    return bass.ds(iv + add, 1), add % 2
                return c, c % 2

            def emit_proj(l, c):
                """Input projection for layer l chunk c. Leaves xp for all 16
                steps resident in PSUM (rows (t,b), has_written set); the scan
                matmuls then accumulate the recurrent term directly onto it."""
                chs, w = chunk_idx(c)
                if l == 0:
                    nc.sync.dma_start(
                        out=xTring[:, :, :, w, :, :], in_=xT[chs, :, :, :, :, :]
                    )
                src = xTring if l == 0 else projring[l - 1]
                pp = pproj.tile([128, D], f32, tag=f"pp{l}", name=f"pp{l}")
                for h in range(2):
                    js = slice(h * 512, (h + 1) * 512)
                    nc.tensor.matmul(
                        pp[:, js],
                        lhsT=ones[:, :],
                        rhs=bias_sb[l][:, js],
                        start=True,
                        stop=False,
                        skip_group_check=True,
                    )
                    for kc in range(KC):
                        lh = src[:, 0, kc, w, :, :]  # [128, CH, B]
                        rh = wih_sb[l][:, kc, 0, js]  # [128, 512]
                        nc.tensor.matmul(
                            pp[:, js],
                            lhsT=lh,
                            rhs=rh,
                            start=False,
                            stop=False,
                            skip_group_check=True,
                        )
                return pp

            def emit_scan_mm(l, t, pp):
                """Accumulate h_{t-1} @ Whh^T onto the xp rows of step t that
                already sit in pp. The stationary is a 32-wide zero-padded
                slot with h^T at column offset (t%4)*8, so the matmul output
                is the 32-aligned row group containing step t's rows; the
                zero columns accumulate 0 onto the other steps' rows."""
                tprev = (t - 1) % CH
                g = t // 4
                for h in range(2):
                    js = slice(h * 512, (h + 1) * 512)
                    for kc in range(KC):
                        lh = scanring[l][:, tprev, 0, kc, :]  # [128, 32]
                        rh = whh_sb[l][:, kc, 0, js]  # [128, 512]
                        nc.tensor.matmul(
                            pp[32 * g : 32 * (g + 1), js],
                            lhsT=lh,
                            rhs=rh,
                            start=False,
                            stop=(t == CH - 1 and h == 1 and kc == KC - 1),
                            skip_group_check=True,
                            tile_position=(0, 32 * g),
                        )

            def emit_act(l, t, c, pp, y_acc):
                """Activation on the full aligned 32-row group; only step t's
                8 rows (offset (t%4)*8) are complete -- the rest is garbage
                that the shifted-selector transpose filters out."""
                func = ACT.Relu if l == 0 else ACT.Tanh
                g = t // 4
                if l == 2:
                    h32 = y_acc[:, t, :]
                else:
                    h32 = hp.tile([32, D], bf16, tag=f"h{l}", name=f"h{l}")
                nc.scalar.activation(
                    h32[:, :], pp[32 * g : 32 * (g + 1), :], func, scale=scale
                )
                return h32

            def emit_transpose(l, t, c, h32):
                """h -> h^T via transpose-mode matmul whose moving operand is
                a shifted-masked selector: valid rows 8*(t%4)+b land at col
                8*((t+1)%4)+b (where step t+1's matmul wants them), garbage
                rows are dropped and the other columns come out zero."""
                r = t % 4
                rn = (t + 1) % 4
                pT = pst.tile([128, KC, 32], bf16, tag="pT")
                for kc in range(KC):
                    nc.tensor.transpose(
                        out=pT[:, kc, :],
                        in_=h32[:, kc * 128 : (kc + 1) * 128],
                        identity=ish[:, r, :],
                    )
                nc.vector.tensor_copy(
                    out=scanring[l][:, t, 0, :, :], in_=pT[:, :, :]
                )
                if l < 2:
                    _, w = chunk_idx(c)
                    nc.vector.tensor_copy(
                        out=projring[l][:, 0, :, w, t, :],
                        in_=pT[:, :, 8 * rn : 8 * (rn + 1)],
                    )

            def emit_iteration(insts):
                """insts: list of (layer, chunk); scans interleaved per step.
                Transposes of layer l are emitted after layer l+1's matmuls so
                the PE reaches them once l's activation has drained."""
                pps = {}
                y_acc = None
                for l, c in insts:
                    pps[l] = emit_proj(l, c)
                    if l == 2:
                        y_acc = ypool.tile([32, CH, D], bf16, tag="yacc", name="yacc")
                for t in range(CH):
                    pend = []
                    for l, c in insts:
                        emit_scan_mm(l, t, pps[l])
                        h_sb = emit_act(l, t, c, pps[l], y_acc)
                        pend.append((l, c, h_sb))
                        if len(pend) > 1:
                            pl, pc, ph = pend.pop(0)
                            emit_transpose(pl, t, pc, ph)
                    for pl, pc, ph in pend:
                        emit_transpose(pl, t, pc, ph)
                for l, c in insts:
                    if l == 2:
                        chs, _ = chunk_idx(c)
                        for r in range(4):
                            dst = y[chs, :, :, :]
                            if isinstance(c, tuple):
                                dst = dst[:, r::4, :, :].rearrange(
                                    "a t b d -> a b t d"
                                )
                            else:
                                dst = dst[r::4, :, :].rearrange("t b d -> b t d")
                            nc.sync.dma_start(
                                out=dst, in_=y_acc[8 * r : 8 * (r + 1), r::4, :]
                            )

            def emit_all():
                emit_iteration([(0, 0)])
                emit_iteration([(0, 1), (1, 0)])
                if unroll:
                    for ivv in range(0, NCH - 2, 2):
                        emit_iteration([(0, ivv + 2), (1, ivv + 1), (2, ivv)])
                        emit_iteration([(0, ivv + 3), (1, ivv + 2), (2, ivv + 1)])
                else:
                    with tc.For_i(
                        0, NCH - 2, 2, hint_engines=(mybir.EngineType.Pool,)
                    ) as iv:
                        emit_iteration([(0, (iv, 2)), (1, (iv, 1)), (2, (iv, 0))])
                        emit_iteration([(0, (iv, 3)), (1, (iv, 2)), (2, (iv, 1))])
                emit_iteration([(1, NCH - 1), (2, NCH - 2)])
                emit_iteration([(2, NCH - 1)])

            if repeat == 1:
                emit_all()
            else:
                with tc.For_i(0, repeat, 1) as _rep:
                    emit_all()
                    for l in range(3):
                        nc.vector.memset(scanring[l], 0.0)

    _BUILD_CACHE[key] = nc
    return nc


def _prep_inputs(x, wihT_l, whhT_l, bias_l, fp8=None):
    """Build per-core in_maps.

    x: [SEQ, BATCH, D] f32; wihT_l/whhT_l: [3, D_in, D_out] f32 (stacked W^T);
    bias_l: [3, D] f32 (bih + bhh).
    """
    import ml_dtypes

    if fp8 is None:
        fp8 = FP8
    bf = ml_dtypes.bfloat16
    wnp = ml_dtypes.float8_e4m3 if fp8 else bf
    KC = 4 if fp8 else 8
    O = 2 if fp8 else 1
    S = WSCALE if fp8 else 1.0

    def packw(wT):
        # wT [3, Din, Dout] -> [3, 128, KC, O, D]
        w = (wT * S).reshape(3, KC, O, 128, D)
        return np.ascontiguousarray(w.transpose(0, 3, 1, 2, 4)).astype(wnp)

    wihp = packw(wihT_l)
    whhp = packw(whhT_l)
    biasp = np.ascontiguousarray((bias_l * S)[:, None, :]).astype(bf)

    ishp = np.zeros((32, 4, 32), dtype=bf)
    for r in range(4):
        for k in range(8 * r, 8 * r + 8):
            ishp[k, r, (k + 8) % 32] = 1.0

    in_maps = []
    for c in range(NCORES):
        xc = x[:, c * B : (c + 1) * B, :]  # [SEQ, B, D]
        xr = xc.reshape(NCH, CH, B, KC, O, 128)
        xr = np.ascontiguousarray(xr.transpose(0, 5, 4, 3, 1, 2))  # ch p o kc t b
        in_maps.append(
            {
                "xT": xr.astype(wnp),
                "wihT": wihp,
                "whhT": whhp,
                "bias": biasp,
                "ishift": ishp,
            }
        )
    return in_maps


def kernel(
    x,
    Wih0,
    Whh0,
    bih0,
    bhh0,
    Wih1,
    Whh1,
    bih1,
    bhh1,
    Wih2,
    Whh2,
    bih2,
    bhh2,
):
    from concourse import bass_utils

    x = np.asarray(x, dtype=np.float32)
    wihT = np.ascontiguousarray(
        np.stack([np.asarray(w, np.float32).T for w in (Wih0, Wih1, Wih2)])
    )
    whhT = np.ascontiguousarray(
        np.stack([np.asarray(w, np.float32).T for w in (Whh0, Whh1, Whh2)])
    )
    bias = np.stack(
        [
            np.asarray(a, np.float32) + np.asarray(b, np.float32)
            for a, b in ((bih0, bhh0), (bih1, bhh1), (bih2, bhh2))
        ]
    )
    nc = build(repeat=1)
    in_maps = _prep_inputs(x, wihT, whhT, bias)
    res = bass_utils.run_bass_kernel_spmd(
        nc, in_maps, core_ids=list(range(NCORES)), trace=False
    )
    out = np.empty((SEQ, BATCH, D), dtype=np.float32)
    for c in range(NCORES):
        yc = res.results[c]["y"].astype(np.float32)  # [NCH, CH, B, D]
        out[:, c * B : (c + 1) * B, :] = yc.reshape(SEQ, B, D)
    return out
